# revision 1
# baseline (speedup 1.0000x reference)
"""Multi-head attention (Keras-style, relu-activated dense projections)
for Trainium2, SPMD across 8 NeuronCores.

Problem (full shapes):
    B, S, D, H = 4, 1024, 1024, 16 ; DH = 64
    qp = relu(q @ Wq + bq); kp = relu(k @ Wk + bk); vp = relu(v @ Wv + bv)
    per head h: scores = qh @ kh^T / 8 ; attn = softmax(scores)
    out = relu(concat_h(attn @ vh) @ Wo + bo)

Sharding: core c = (batch b = c//2, head-group g = c%2). Each core computes
the 8 heads of group g for batch b end-to-end and produces the partial
output projection  attn_out_g @ Wo[g*512:(g+1)*512, :]  (no bias / relu).
Host sums the two partials per batch, adds bo, applies relu.

Per-core dataflow (head pair hp = heads 2hp / 2hp+1):
  - host feeds q[b].T etc so projections contract d on the partition dim.
  - Q/K projections transposed: qpT/kpT [128, 4(hp), 1024(s)]; head 2hp at
    partitions 0:64, head 2hp+1 at 64:128 -> the K=64 score matmuls of a
    pair auto-land in different PE row groups and run concurrently.
  - scores pair writes one [128, 1024] 2-bank PSUM tile; one wide exp (ACT)
    emits ex [128, ut, 1024] bf16 (head A cols 0:512, B 512:1024).
  - attn@v: column-paired bf16 matmuls into nt[0:64] / nt[64:128].
  - softmax denominator: DVE tree-sums ex over ut, two K=128 matmuls with a
    ones column reduce partitions -> Z_A (psum row 0) / Z_B (row 32); a
    masked K=33 matmul broadcasts both to [128, 512]; wide DVE reciprocal +
    a single [128, 512] multiply writes attn_out.
  - output projection: full K=128 accumulating matmuls over head pairs.
  - matmuls in float32r (fp22, full PE rate) except the bf16 attention core.
"""

import numpy as np
from contextlib import ExitStack

import concourse.bass as bass
import concourse.mybir as mybir
import concourse.tile as tile
from concourse import bacc

# ---- constants (hardcoded per the contract; kernel.py must be self-contained)
B, S, D, H = 4, 1024, 1024, 16
DG = 512          # feature slice per core (8 heads)
HL = 8            # heads per core
DH = 64
P = 128
NCORES = 8
NJT = DG // P     # 4 feature tiles == head pairs
NST = S // P      # 8 sequence tiles
NDT = D // P      # 8 contraction tiles for projections
NPC = S // 512    # 2 query chunks of 512

F32 = mybir.dt.float32
F32R = mybir.dt.float32r
BF16 = mybir.dt.bfloat16
AF = mybir.ActivationFunctionType


def _d(ap):
    """View a float32 DRAM AP as float32r so DMAs into f32r tiles type-check.
    (walrus requires fp32r matmul operands to be *produced* as fp32r.)"""
    return ap.bitcast(F32R)


def build_bass():
    nc = bacc.Bacc("TRN2", target_bir_lowering=False, debug=False,
                   num_devices=NCORES)

    xqT = nc.dram_tensor("xqT", [D, S], F32, kind="ExternalInput").ap()
    xkT = nc.dram_tensor("xkT", [D, S], F32, kind="ExternalInput").ap()
    xvT = nc.dram_tensor("xvT", [D, S], F32, kind="ExternalInput").ap()
    wq = nc.dram_tensor("wq", [D, DG], F32, kind="ExternalInput").ap()
    wk = nc.dram_tensor("wk", [D, DG], F32, kind="ExternalInput").ap()
    wv = nc.dram_tensor("wv", [D, DG], F32, kind="ExternalInput").ap()
    bq = nc.dram_tensor("bq", [1, DG], F32, kind="ExternalInput").ap()
    bk = nc.dram_tensor("bk", [1, DG], F32, kind="ExternalInput").ap()
    bv = nc.dram_tensor("bv", [1, DG], F32, kind="ExternalInput").ap()
    wo = nc.dram_tensor("wo", [DG, D], F32, kind="ExternalInput").ap()
    ones_in = nc.dram_tensor("ones", [1, 512], F32, kind="ExternalInput").ap()
    bcm_in = nc.dram_tensor("bcmask", [33, P], F32, kind="ExternalInput").ap()
    out = nc.dram_tensor("out", [S, D], F32, kind="ExternalOutput").ap()

    with tile.TileContext(nc) as tc, ExitStack() as ctx, \
            nc.allow_low_precision(reason="fp32r/bf16 compute is intentional"):
        consts = ctx.enter_context(tc.tile_pool(name="consts", bufs=1))
        xpool = ctx.enter_context(tc.tile_pool(name="xpool", bufs=20))
        wpool = ctx.enter_context(tc.tile_pool(name="wpool", bufs=16))
        qkpool = ctx.enter_context(tc.tile_pool(name="qkpool", bufs=1))
        vpool = ctx.enter_context(tc.tile_pool(name="vpool", bufs=1))
        epool = ctx.enter_context(tc.tile_pool(name="epool", bufs=2))
        aopool = ctx.enter_context(tc.tile_pool(name="aopool", bufs=1))
        t1pool = ctx.enter_context(tc.tile_pool(name="t1pool", bufs=1))
        espool = ctx.enter_context(tc.tile_pool(name="espool", bufs=2))
        rpool = ctx.enter_context(tc.tile_pool(name="rpool", bufs=2))
        outpool = ctx.enter_context(tc.tile_pool(name="outpool", bufs=3))

        psA = ctx.enter_context(tc.tile_pool(name="psA", bufs=2, space="PSUM"))
        psB = ctx.enter_context(tc.tile_pool(name="psB", bufs=2, space="PSUM"))
        psZ = ctx.enter_context(tc.tile_pool(name="psZ", bufs=1, space="PSUM"))
        psD = ctx.enter_context(tc.tile_pool(name="psD", bufs=1, space="PSUM"))

        # --- constants
        ones = consts.tile([P, 512], F32R, tag="ones")
        nc.sync.dma_start(out=ones, in_=_d(ones_in.to_broadcast([P, 512])))
        onescol = consts.tile([P, 1], BF16, tag="onescol")
        nc.vector.memset(onescol, 1.0)
        bcmask = consts.tile([33, P], F32R, tag="bcmask")
        nc.sync.dma_start(out=bcmask, in_=_d(bcm_in))
        # zsb: persistent Z staging rows (0 and 32); fill once with finite
        # values so the masked K=33 broadcast matmul never reads NaNs.
        zsb = consts.tile([33, 512], F32R, tag="zsb")
        nc.sync.dma_start(out=zsb, in_=_d(ones_in.to_broadcast([33, 512])))

        bv_sb = consts.tile([1, DG], F32R, tag="bv")
        nc.sync.dma_start(out=bv_sb, in_=_d(bv))

        # --- transposed projections for Q and K
        qpT = qkpool.tile([P, NJT, S], F32R, tag="qpT")
        kpT = qkpool.tile([P, NJT, S], F32R, tag="kpT")

        # per-partition bias for the transposed projections (ACT bias input)
        bqT = consts.tile([P, NJT], F32, tag="bqT")
        nc.sync.dma_start(out=bqT, in_=bq[0, :].rearrange("(jt p) -> p jt", p=P))
        bkT = consts.tile([P, NJT], F32, tag="bkT")
        nc.sync.dma_start(out=bkT, in_=bk[0, :].rearrange("(jt p) -> p jt", p=P))

        def load_halves(xT, w):
            xmap = {}
            for pc in range(NPC):
                for dt_ in range(NDT):
                    xt = xpool.tile([P, 512], F32R, tag="xT")
                    nc.sync.dma_start(
                        out=xt,
                        in_=_d(xT[dt_ * P:(dt_ + 1) * P,
                                  pc * 512:(pc + 1) * 512]))
                    xmap[(dt_, pc)] = xt
            wts = []
            for dt_ in range(NDT):
                wt = wpool.tile([P, DG], F32R, tag="w")
                nc.sync.dma_start(out=wt, in_=_d(w[dt_ * P:(dt_ + 1) * P, :]))
                wts.append(wt)
            return xmap, wts

        for name, xT, w, bT, dst in (("q", xqT, wq, bqT, qpT),
                                     ("k", xkT, wk, bkT, kpT)):
            xmap, wts = load_halves(xT, w)
            for pc in range(NPC):
                for jt in range(NJT):
                    ps = psA.tile([P, 1024], F32, tag="ps")
                    half = ps[:, 0:512]
                    for dt_ in range(NDT):
                        nc.tensor.matmul(
                            half,
                            lhsT=wts[dt_][:, jt * P:(jt + 1) * P],
                            rhs=xmap[(dt_, pc)],
                            start=(dt_ == 0), stop=(dt_ == NDT - 1))
                    nc.scalar.activation(
                        dst[:, jt, pc * 512:(pc + 1) * 512], half, AF.Relu,
                        bias=bT[:, jt:jt + 1])

        # --- V projection, natural layout -> vpa [128, st, 512] bf16
        vpa = vpool.tile([P, NST, DG], BF16, tag="vpa")
        xmap, wts = load_halves(xvT, wv)
        for st in range(NST):
            ps = psA.tile([P, 1024], F32, tag="ps")
            half = ps[:, 0:512]
            for dt_ in range(NDT):
                nc.tensor.matmul(
                    half,
                    lhsT=xmap[(dt_, st // 4)][:, (st % 4) * P:(st % 4 + 1) * P],
                    rhs=wts[dt_],
                    start=(dt_ == 0), stop=False)
            nc.tensor.matmul(
                half, lhsT=ones[0:1, 0:P], rhs=bv_sb,
                start=False, stop=True)
            nc.scalar.activation(vpa[:, st, :], half, AF.Relu)

        # --- attention, one head pair x one 512-query chunk at a time.
        # pc outer: all head pairs of a query chunk finish together, so the
        # matching half of the output projection can start while the second
        # chunk's attention is still running.
        aoT3 = aopool.tile([P, NJT, S], F32R, tag="aoT3")

        # Wo by head pair (emitted here so its DMA runs during attention)
        wo3 = consts.tile([P, NJT, D], F32R, tag="wo3")
        for hp in range(NJT):
            nc.sync.dma_start(out=wo3[:, hp, :],
                              in_=_d(wo[hp * P:(hp + 1) * P, :]))

        for pc in range(NPC):
            pslice = slice(pc * 512, (pc + 1) * 512)
            for hp in range(NJT):
                hA, hB = 2 * hp, 2 * hp + 1
                ex = epool.tile([P, NST, 1024], BF16, tag="exp")
                for ut in range(NST):
                    uslice = slice(ut * P, (ut + 1) * P)
                    pw = psA.tile([P, 1024], F32, tag="ps")
                    nc.tensor.matmul(
                        pw[:, 0:512],
                        lhsT=kpT[0:DH, hp, uslice],
                        rhs=qpT[0:DH, hp, pslice],
                        start=True, stop=True)
                    nc.tensor.matmul(
                        pw[:, 512:1024],
                        lhsT=kpT[DH:P, hp, uslice],
                        rhs=qpT[DH:P, hp, pslice],
                        start=True, stop=True)
                    nc.scalar.activation(ex[:, ut, :], pw, AF.Exp, scale=0.125)
                # Z tree-sum over ut on DVE (overlaps the attn@v matmuls)
                t1 = t1pool.tile([P, 4, 1024], BF16, tag="t1")
                nc.vector.tensor_add(t1, ex[:, 0:4, :], ex[:, 4:8, :])
                nc.vector.tensor_add(t1[:, 0:2, :], t1[:, 0:2, :],
                                     t1[:, 2:4, :])
                exsum = espool.tile([P, 1024], BF16, tag="exsum")
                nc.vector.tensor_add(exsum, t1[:, 0, :], t1[:, 1, :])
                # Z_A -> psum row 0, Z_B -> psum row 32 (col group 1), then
                # stage into SBUF; emitted before attn@v so the copies are
                # long done when PE reaches the broadcast matmul.
                zps = psZ.tile([P, 512], F32, tag="z")
                nc.tensor.matmul(zps[0:1, :], lhsT=onescol,
                                 rhs=exsum[:, 0:512], start=True, stop=True)
                nc.tensor.matmul(zps[32:33, :], lhsT=onescol,
                                 rhs=exsum[:, 512:1024], start=True, stop=True)
                nc.vector.tensor_copy(zsb[0:1, :], zps[0:1, :])
                nc.vector.tensor_copy(zsb[32:33, :], zps[32:33, :])
                # attn @ v: column-paired accumulation over key tiles
                nt = psB.tile([P, 512], F32, tag="nt")
                for ut in range(NST):
                    nc.tensor.matmul(
                        nt[0:DH, :],
                        lhsT=vpa[:, ut, hA * DH:(hA + 1) * DH],
                        rhs=ex[:, ut, 0:512],
                        start=(ut == 0), stop=(ut == NST - 1),
                        skip_group_check=True)
                    nc.tensor.matmul(
                        nt[DH:P, :],
                        lhsT=vpa[:, ut, hB * DH:(hB + 1) * DH],
                        rhs=ex[:, ut, 512:1024],
                        start=(ut == 0), stop=(ut == NST - 1),
                        skip_group_check=True)
                # broadcast: rows 0:64 <- Z_A, rows 64:128 <- Z_B
                zbc = psZ.tile([P, 512], F32, tag="z")
                nc.tensor.matmul(zbc, lhsT=bcmask, rhs=zsb,
                                 start=True, stop=True)
                rcp = rpool.tile([P, 512], F32, tag="rcp")
                nc.vector.reciprocal_approx_fast(rcp, zbc)
                nc.vector.tensor_mul(aoT3[:, hp, pslice], nt, rcp)

            # output projection for this query chunk (pt = pc*4 .. pc*4+3)
            for pt in range(pc * 4, pc * 4 + 4):
                for jj in range(2):
                    po_ = psD.tile([P, 512], F32, tag="po")
                    for hp in range(NJT):
                        nc.tensor.matmul(
                            po_,
                            lhsT=aoT3[:, hp, pt * P:(pt + 1) * P],
                            rhs=wo3[:, hp, jj * 512:(jj + 1) * 512],
                            start=(hp == 0), stop=(hp == NJT - 1))
                    os_ = outpool.tile([P, 512], F32, tag="os")
                    nc.vector.tensor_copy(os_, po_)
                    nc.sync.dma_start(
                        out=out[pt * P:(pt + 1) * P, jj * 512:(jj + 1) * 512],
                        in_=os_)

    nc.compile()
    return nc


_CACHE = {}


def get_nc():
    if "nc" not in _CACHE:
        _CACHE["nc"] = build_bass()
    return _CACHE["nc"]


def make_bcmask():
    m = np.zeros((33, P), np.float32)
    m[0, 0:DH] = 1.0
    m[32, DH:P] = 1.0
    return m


def make_in_maps(q, k, v, Wq, bq, Wk, bk, Wv, bv, Wo, bo):
    q = np.asarray(q, np.float32)
    k = np.asarray(k, np.float32)
    v = np.asarray(v, np.float32)
    Wq = np.asarray(Wq, np.float32)
    Wk = np.asarray(Wk, np.float32)
    Wv = np.asarray(Wv, np.float32)
    Wo = np.asarray(Wo, np.float32)
    bq = np.asarray(bq, np.float32)
    bk = np.asarray(bk, np.float32)
    bv = np.asarray(bv, np.float32)

    qT = [np.ascontiguousarray(q[b].T) for b in range(B)]
    kT = [np.ascontiguousarray(k[b].T) for b in range(B)]
    vT = [np.ascontiguousarray(v[b].T) for b in range(B)]
    bcm = make_bcmask()

    in_maps = []
    for c in range(NCORES):
        b, g = divmod(c, 2)
        sl = slice(g * DG, (g + 1) * DG)
        in_maps.append({
            "xqT": qT[b],
            "xkT": kT[b],
            "xvT": vT[b],
            "wq": np.ascontiguousarray(Wq[:, sl]),
            "wk": np.ascontiguousarray(Wk[:, sl]),
            "wv": np.ascontiguousarray(Wv[:, sl]),
            "bq": np.ascontiguousarray(bq[sl]).reshape(1, DG),
            "bk": np.ascontiguousarray(bk[sl]).reshape(1, DG),
            "bv": np.ascontiguousarray(bv[sl]).reshape(1, DG),
            "wo": np.ascontiguousarray(Wo[sl, :]),
            "ones": np.ones((1, 512), np.float32),
            "bcmask": bcm,
        })
    return in_maps


def combine_outputs(parts, bo):
    bo = np.asarray(bo, np.float32)
    out = np.empty((B, S, D), np.float32)
    for b in range(B):
        out[b] = np.maximum(parts[2 * b] + parts[2 * b + 1] + bo[None, :], 0.0)
    return out


def run(in_maps, trace=False, **kwargs):
    from concourse.bass_utils import run_bass_kernel_spmd
    nc = get_nc()
    return run_bass_kernel_spmd(nc, in_maps, list(range(NCORES)),
                                trace=trace, **kwargs)


def kernel(q, k, v, Wq, bq, Wk, bk, Wv, bv, Wo, bo):
    in_maps = make_in_maps(q, k, v, Wq, bq, Wk, bk, Wv, bv, Wo, bo)
    res = run(in_maps)
    parts = [res.results[c]["out"] for c in range(NCORES)]
    return combine_outputs(parts, bo)



# revision 4
# speedup vs baseline: 1.4370x; 1.4370x over previous
"""Multi-head attention (Keras-style, relu-activated dense projections)
for Trainium2, SPMD across 8 NeuronCores.

Problem (full shapes):
    B, S, D, H = 4, 1024, 1024, 16 ; DH = 64
    qp = relu(q @ Wq + bq); kp = relu(k @ Wk + bk); vp = relu(v @ Wv + bv)
    per head h: scores = qh @ kh^T / 8 ; attn = softmax(scores)
    out = relu(concat_h(attn @ vh) @ Wo + bo)

Sharding: core c = (batch b = c//2, head-group g = c%2). Each core computes
the 8 heads of group g for batch b end-to-end and produces the partial
output projection  attn_out_g @ Wo[g*512:(g+1)*512, :]  (no bias / relu).
Host sums the two partials per batch, adds bo, applies relu.

v2 design (vs the f32r baseline): everything bf16 (halved DMA, no fp32
moving-operand limits), and the emission order is software-pipelined so
the PE never idles long enough for the HAM clock gate to re-throttle:

  - unit U = (pc query-chunk, hp head-pair), pc-major order.
  - scores for unit U are emitted as 8 ut-chunks (2 row-group-paired
    K=64 matmuls each) with "filler" PE work interleaved between chunks:
    the remaining Q/K projections, the V projection, attn@V of earlier
    units, and the pc=0 output projection.  The psA PSUM ring (2 tiles)
    paces scores to the ACT exp drain; the filler keeps the PE busy
    during those waits.
  - attn@V of unit U is gated per-ut on exp(U, ut) and column-pair
    packed (M=64 col groups).  Z = sum_keys exp: DVE tree-sum over ut,
    two M=1 matmuls (partitions 0/32), masked K=33 broadcast matmul,
    reciprocal, one [128,512] multiply.
  - Q/K projection relu+bias on ACT (per-partition bias); V projection
    relu on DVE (tensor_scalar_max) with bias via a K=1 ones matmul.
"""

import numpy as np
from contextlib import ExitStack

import ml_dtypes

import concourse.bass as bass
import concourse.mybir as mybir
import concourse.tile as tile
from concourse import bacc

BF16NP = ml_dtypes.bfloat16

# ---- constants (hardcoded per the contract; kernel.py must be self-contained)
B, S, D, H = 4, 1024, 1024, 16
DG = 512          # feature slice per core (8 heads)
DH = 64
P = 128
NCORES = 8
NJT = DG // P     # 4 feature tiles == head pairs
NST = S // P      # 8 sequence tiles
NDT = D // P      # 8 contraction tiles for projections
NPC = S // 512    # 2 query chunks of 512

F32 = mybir.dt.float32
BF16 = mybir.dt.bfloat16
AF = mybir.ActivationFunctionType


def build_bass():
    nc = bacc.Bacc("TRN2", target_bir_lowering=False, debug=False,
                   num_devices=NCORES)

    xqT = nc.dram_tensor("xqT", [D, S], BF16, kind="ExternalInput").ap()
    xkT = nc.dram_tensor("xkT", [D, S], BF16, kind="ExternalInput").ap()
    xvT = nc.dram_tensor("xvT", [D, S], BF16, kind="ExternalInput").ap()
    wq = nc.dram_tensor("wq", [D, DG], BF16, kind="ExternalInput").ap()
    wk = nc.dram_tensor("wk", [D, DG], BF16, kind="ExternalInput").ap()
    wv = nc.dram_tensor("wv", [D, DG], BF16, kind="ExternalInput").ap()
    bq = nc.dram_tensor("bq", [1, DG], F32, kind="ExternalInput").ap()
    bk = nc.dram_tensor("bk", [1, DG], F32, kind="ExternalInput").ap()
    bv = nc.dram_tensor("bv", [1, DG], BF16, kind="ExternalInput").ap()
    wo = nc.dram_tensor("wo", [DG, D], BF16, kind="ExternalInput").ap()
    bcm_in = nc.dram_tensor("bcmask", [33, P], BF16, kind="ExternalInput").ap()
    out = nc.dram_tensor("out", [S, D], F32, kind="ExternalOutput").ap()

    # unit order: pc-major so the pc=0 output projection can start while
    # pc=1 attention is still draining.
    UNITS = [(pc, hp) for pc in range(NPC) for hp in range(NJT)]

    with tile.TileContext(nc) as tc, ExitStack() as ctx, \
            nc.allow_low_precision(reason="bf16 compute is intentional"):
        consts = ctx.enter_context(tc.tile_pool(name="consts", bufs=1))
        # x inputs and ex tiles share one ring: identical [P, 8, 1024] bf16
        # shape, and the x tiles die exactly when the later ex tiles need
        # their space.
        bigpool = ctx.enter_context(tc.tile_pool(name="bigpool", bufs=7))
        wpool = ctx.enter_context(tc.tile_pool(name="wpool", bufs=3))
        wopool = ctx.enter_context(tc.tile_pool(name="wopool", bufs=1))
        qkpool = ctx.enter_context(tc.tile_pool(name="qkpool", bufs=1))
        vpool = ctx.enter_context(tc.tile_pool(name="vpool", bufs=1))
        t1pool = ctx.enter_context(tc.tile_pool(name="t1pool", bufs=1))
        espool = ctx.enter_context(tc.tile_pool(name="espool", bufs=2))
        rpool = ctx.enter_context(tc.tile_pool(name="rpool", bufs=2))
        aopool = ctx.enter_context(tc.tile_pool(name="aopool", bufs=1))
        outpool = ctx.enter_context(tc.tile_pool(name="outpool", bufs=2))

        psA = ctx.enter_context(tc.tile_pool(name="psA", bufs=2, space="PSUM"))
        psP = ctx.enter_context(tc.tile_pool(name="psP", bufs=1, space="PSUM"))
        psB = ctx.enter_context(tc.tile_pool(name="psB", bufs=1, space="PSUM"))
        psZ = ctx.enter_context(tc.tile_pool(name="psZ", bufs=1, space="PSUM"))

        # --- constants (tiny DMAs / memsets first)
        bqT = consts.tile([P, NJT], F32, tag="bqT")
        nc.sync.dma_start(out=bqT, in_=bq[0, :].rearrange("(jt p) -> p jt", p=P))
        bkT = consts.tile([P, NJT], F32, tag="bkT")
        nc.sync.dma_start(out=bkT, in_=bk[0, :].rearrange("(jt p) -> p jt", p=P))
        bv_sb = consts.tile([1, DG], BF16, tag="bv")
        nc.sync.dma_start(out=bv_sb, in_=bv)
        bcmask = consts.tile([33, P], BF16, tag="bcmask")
        nc.sync.dma_start(out=bcmask, in_=bcm_in)
        onescol = consts.tile([P, 1], BF16, tag="onescol")
        nc.vector.memset(onescol, 1.0)
        onesrow = consts.tile([1, P], BF16, tag="onesrow")
        nc.vector.memset(onesrow, 1.0)
        # zsb: persistent Z staging rows (0 and 32); fill once with finite
        # values so the masked K=33 broadcast matmul never reads NaNs.
        zsb = consts.tile([33, 512], BF16, tag="zsb")
        nc.vector.memset(zsb, 1.0)

        # --- big input DMAs, in consumption order.  x tensors split in two
        # S-halves so the first projection matmuls can start earlier.
        def load_x(xT):
            xt = bigpool.tile([P, NDT, S], BF16, tag="big")
            for h in range(2):
                sl = slice(h * 512, (h + 1) * 512)
                nc.sync.dma_start(
                    out=xt[:, :, sl],
                    in_=xT[:, sl].rearrange("(dt p) s -> p dt s", p=P))
            return xt

        def load_w(w):
            wt = wpool.tile([P, NDT, DG], BF16, tag="w")
            nc.sync.dma_start(out=wt, in_=w.rearrange("(dt p) n -> p dt n", p=P))
            return wt

        wqs = load_w(wq)
        xq = load_x(xqT)
        wks = load_w(wk)
        xk = load_x(xkT)
        wvs = load_w(wv)
        xv = load_x(xvT)
        wo3 = wopool.tile([P, NJT, D], BF16, tag="wo3")
        nc.sync.dma_start(out=wo3, in_=wo.rearrange("(hp p) n -> p hp n", p=P))

        qpT = qkpool.tile([P, NJT, S], BF16, tag="qpT")
        kpT = qkpool.tile([P, NJT, S], BF16, tag="kpT")
        vpa = vpool.tile([P, NST, DG], BF16, tag="vpa")
        aoT3 = aopool.tile([P, NJT, S], BF16, tag="aoT3")

        # ---- chunk emitters ------------------------------------------------
        def qk_half(which, hp, pc):
            """8 accumulating matmuls for one pc-half of qpT/kpT pair hp;
            emits the wide relu after the second half."""
            xt, wt, bT, dst = ((xq, wqs, bqT, qpT) if which == "q"
                              else (xk, wks, bkT, kpT))
            if pc == 0:
                ps = qk_half.ps = psP.tile([P, 1024], F32, tag="ps", name="ps")
            else:
                ps = qk_half.ps
            half = ps[:, pc * 512:(pc + 1) * 512]
            for dt_ in range(NDT):
                nc.tensor.matmul(
                    half,
                    lhsT=wt[:, dt_, hp * P:(hp + 1) * P],
                    rhs=xt[:, dt_, pc * 512:(pc + 1) * 512],
                    start=(dt_ == 0), stop=(dt_ == NDT - 1))
            if pc == 1:
                nc.scalar.activation(dst[:, hp, :], ps, AF.Relu,
                                     bias=bT[:, hp:hp + 1])

        def v_chunk(st):
            """V projection for sequence tile st: 8 matmuls + bias matmul,
            relu on DVE (keeps ACT free for exp)."""
            ps = psP.tile([P, 1024], F32, tag="ps")
            half = ps[:, 0:512]
            for dt_ in range(NDT):
                nc.tensor.matmul(
                    half,
                    lhsT=xv[:, dt_, st * P:(st + 1) * P],
                    rhs=wvs[:, dt_, :],
                    start=(dt_ == 0), stop=False)
            nc.tensor.matmul(half, lhsT=onesrow, rhs=bv_sb,
                             start=False, stop=True)
            nc.vector.tensor_scalar_max(vpa[:, st, :], half, 0.0)

        ex_tiles = {}

        def sc_chunk(u, ut):
            """Scores ut-tile of unit u: 2 concurrent K=64 matmuls (head
            pair in row groups 0:64 / 64:128) + the exp into ex."""
            pc, hp = UNITS[u]
            if ut == 0:
                ex_tiles[u] = bigpool.tile([P, NST, 1024], BF16, tag="big", name="ex")
            ex = ex_tiles[u]
            pslice = slice(pc * 512, (pc + 1) * 512)
            uslice = slice(ut * P, (ut + 1) * P)
            pw = psA.tile([P, 1024], F32, tag="ps")
            nc.tensor.matmul(
                pw[:, 0:512],
                lhsT=kpT[0:DH, hp, uslice],
                rhs=qpT[0:DH, hp, pslice],
                start=True, stop=True)
            nc.tensor.matmul(
                pw[:, 512:1024],
                lhsT=kpT[DH:P, hp, uslice],
                rhs=qpT[DH:P, hp, pslice],
                start=True, stop=True)
            nc.scalar.activation(ex[:, ut, :], pw, AF.Exp, scale=0.125)

        def tree_chunk(u):
            """DVE tree-sum of exp over the 8 ut tiles -> exsum [128,1024]."""
            ex = ex_tiles[u]
            t1 = t1pool.tile([P, 4, 1024], BF16, tag="t1")
            nc.vector.tensor_add(t1, ex[:, 0:4, :], ex[:, 4:8, :])
            nc.vector.tensor_add(t1[:, 0:2, :], t1[:, 0:2, :], t1[:, 2:4, :])
            exsum = espool.tile([P, 1024], BF16, tag="exsum")
            nc.vector.tensor_add(exsum, t1[:, 0, :], t1[:, 1, :])
            return exsum

        nt_tiles = {}

        def av_chunk(u, uts):
            """attn@V for unit u over ut in uts: column-pair-packed bf16
            matmuls accumulating into nt (head A rows 0:64, head B 64:128)."""
            pc, hp = UNITS[u]
            hA, hB = 2 * hp, 2 * hp + 1
            ex = ex_tiles[u]
            if uts[0] == 0:
                nt_tiles[u] = psB.tile([P, 512], F32, tag="nt", name="nt")
            nt = nt_tiles[u]
            for ut in uts:
                nc.tensor.matmul(
                    nt[0:DH, :],
                    lhsT=vpa[:, ut, hA * DH:(hA + 1) * DH],
                    rhs=ex[:, ut, 0:512],
                    start=(ut == 0), stop=(ut == NST - 1),
                    skip_group_check=True)
                nc.tensor.matmul(
                    nt[DH:P, :],
                    lhsT=vpa[:, ut, hB * DH:(hB + 1) * DH],
                    rhs=ex[:, ut, 512:1024],
                    start=(ut == 0), stop=(ut == NST - 1),
                    skip_group_check=True)

        def z_chunk(u, exsum):
            """Z reduction + broadcast + reciprocal + normalize for unit u:
            writes aoT3[:, hp, pslice]."""
            pc, hp = UNITS[u]
            pslice = slice(pc * 512, (pc + 1) * 512)
            nt = nt_tiles.pop(u)
            zps = psZ.tile([P, 512], F32, tag="z")
            nc.tensor.matmul(zps[0:1, :], lhsT=onescol,
                             rhs=exsum[:, 0:512], start=True, stop=True)
            nc.tensor.matmul(zps[32:33, :], lhsT=onescol,
                             rhs=exsum[:, 512:1024], start=True, stop=True)
            nc.vector.tensor_copy(zsb[0:1, :], zps[0:1, :])
            nc.vector.tensor_copy(zsb[32:33, :], zps[32:33, :])
            zbc = psZ.tile([P, 512], F32, tag="z")
            nc.tensor.matmul(zbc, lhsT=bcmask, rhs=zsb, start=True, stop=True)
            rcp = rpool.tile([P, 512], F32, tag="rcp")
            nc.vector.reciprocal_approx_fast(rcp, zbc)
            nc.vector.tensor_mul(aoT3[:, hp, pslice], nt, rcp)
            del ex_tiles[u]

        def outp_chunk(pc, pt):
            """Output projection for query tile pt of chunk pc: 2x4
            accumulating matmuls, one wide copy, one DMA."""
            po = psP.tile([P, 1024], F32, tag="ps")
            for jj in range(2):
                for hp in range(NJT):
                    nc.tensor.matmul(
                        po[:, jj * 512:(jj + 1) * 512],
                        lhsT=aoT3[:, hp, pt * P:(pt + 1) * P],
                        rhs=wo3[:, hp, jj * 512:(jj + 1) * 512],
                        start=(hp == 0), stop=(hp == NJT - 1))
            os_ = outpool.tile([P, 1024], F32, tag="os")
            nc.vector.tensor_copy(os_, po)
            nc.sync.dma_start(out=out[pt * P:(pt + 1) * P, :], in_=os_)

        # ---- emission schedule --------------------------------------------
        def stretch(u, fillers):
            """Scores unit u (8 ut chunks) with filler chunks interleaved
            2 score-chunks : 1 filler."""
            fi = iter(fillers)
            for ut in range(NST):
                sc_chunk(u, ut)
                if ut % 2 == 1:
                    f = next(fi, None)
                    if f is not None:
                        f()
            for f in fi:
                f()

        # head-pair 0 projections first (nothing to overlap them with)
        for pc in range(NPC):
            qk_half("q", 0, pc)
        for pc in range(NPC):
            qk_half("k", 0, pc)

        # U0..U3 = pc0 units; U4..U7 = pc1 units
        stretch(0, [lambda: qk_half("q", 1, 0), lambda: qk_half("q", 1, 1),
                    lambda: qk_half("k", 1, 0), lambda: qk_half("k", 1, 1)])
        stretch(1, [lambda: qk_half("q", 2, 0), lambda: qk_half("q", 2, 1),
                    lambda: qk_half("k", 2, 0), lambda: qk_half("k", 2, 1)])
        stretch(2, [lambda: qk_half("q", 3, 0), lambda: qk_half("q", 3, 1),
                    lambda: qk_half("k", 3, 0), lambda: qk_half("k", 3, 1)])
        stretch(3, [lambda: v_chunk(0), lambda: v_chunk(1),
                    lambda: v_chunk(2), lambda: v_chunk(3)])
        stretch(4, [lambda: v_chunk(4), lambda: v_chunk(5),
                    lambda: v_chunk(6), lambda: v_chunk(7)])
        # exp(U0) finished during stretch 1; attn@V of U0/U1 fills 5/6.
        es = {}
        es[0] = tree_chunk(0)
        stretch(5, [lambda: av_chunk(0, (0, 1)), lambda: av_chunk(0, (2, 3)),
                    lambda: av_chunk(0, (4, 5)), lambda: av_chunk(0, (6, 7)),
                    lambda: z_chunk(0, es[0])])
        es[1] = tree_chunk(1)
        stretch(6, [lambda: av_chunk(1, (0, 1)), lambda: av_chunk(1, (2, 3)),
                    lambda: av_chunk(1, (4, 5)), lambda: av_chunk(1, (6, 7)),
                    lambda: z_chunk(1, es[1])])
        es[2] = tree_chunk(2)
        stretch(7, [lambda: av_chunk(2, (0, 1)), lambda: av_chunk(2, (2, 3)),
                    lambda: av_chunk(2, (4, 5)), lambda: av_chunk(2, (6, 7)),
                    lambda: z_chunk(2, es[2])])

        # tail: remaining attn@V units zipped with the output projections.
        es[3] = tree_chunk(3)
        av_chunk(3, (0, 1, 2, 3))
        av_chunk(3, (4, 5, 6, 7))
        z_chunk(3, es[3])
        # pc0 attention fully resolved -> outproj(pc0) interleaves with the
        # pc1 attn@V drain.
        es[4] = tree_chunk(4)
        av_chunk(4, (0, 1, 2, 3))
        outp_chunk(0, 0)
        av_chunk(4, (4, 5, 6, 7))
        z_chunk(4, es[4])
        es[5] = tree_chunk(5)
        av_chunk(5, (0, 1, 2, 3))
        outp_chunk(0, 1)
        av_chunk(5, (4, 5, 6, 7))
        z_chunk(5, es[5])
        es[6] = tree_chunk(6)
        av_chunk(6, (0, 1, 2, 3))
        outp_chunk(0, 2)
        av_chunk(6, (4, 5, 6, 7))
        z_chunk(6, es[6])
        es[7] = tree_chunk(7)
        av_chunk(7, (0, 1, 2, 3))
        outp_chunk(0, 3)
        av_chunk(7, (4, 5, 6, 7))
        z_chunk(7, es[7])
        for pt in range(4):
            outp_chunk(1, 4 + pt)

    nc.compile()
    return nc


_CACHE = {}


def get_nc():
    if "nc" not in _CACHE:
        _CACHE["nc"] = build_bass()
    return _CACHE["nc"]


def make_bcmask():
    m = np.zeros((33, P), np.float32)
    m[0, 0:DH] = 1.0
    m[32, DH:P] = 1.0
    return m.astype(BF16NP)


def make_in_maps(q, k, v, Wq, bq, Wk, bk, Wv, bv, Wo, bo):
    q = np.asarray(q, np.float32)
    k = np.asarray(k, np.float32)
    v = np.asarray(v, np.float32)
    Wq = np.asarray(Wq, np.float32)
    Wk = np.asarray(Wk, np.float32)
    Wv = np.asarray(Wv, np.float32)
    Wo = np.asarray(Wo, np.float32)
    bq = np.asarray(bq, np.float32)
    bk = np.asarray(bk, np.float32)
    bv = np.asarray(bv, np.float32)

    qT = [np.ascontiguousarray(q[b].T).astype(BF16NP) for b in range(B)]
    kT = [np.ascontiguousarray(k[b].T).astype(BF16NP) for b in range(B)]
    vT = [np.ascontiguousarray(v[b].T).astype(BF16NP) for b in range(B)]
    bcm = make_bcmask()

    in_maps = []
    for c in range(NCORES):
        b, g = divmod(c, 2)
        sl = slice(g * DG, (g + 1) * DG)
        in_maps.append({
            "xqT": qT[b],
            "xkT": kT[b],
            "xvT": vT[b],
            "wq": np.ascontiguousarray(Wq[:, sl]).astype(BF16NP),
            "wk": np.ascontiguousarray(Wk[:, sl]).astype(BF16NP),
            "wv": np.ascontiguousarray(Wv[:, sl]).astype(BF16NP),
            "bq": np.ascontiguousarray(bq[sl]).reshape(1, DG),
            "bk": np.ascontiguousarray(bk[sl]).reshape(1, DG),
            "bv": np.ascontiguousarray(bv[sl]).reshape(1, DG).astype(BF16NP),
            "wo": np.ascontiguousarray(Wo[sl, :]).astype(BF16NP),
            "bcmask": bcm,
        })
    return in_maps


def combine_outputs(parts, bo):
    bo = np.asarray(bo, np.float32)
    out = np.empty((B, S, D), np.float32)
    for b in range(B):
        out[b] = np.maximum(
            np.asarray(parts[2 * b], np.float32)
            + np.asarray(parts[2 * b + 1], np.float32) + bo[None, :], 0.0)
    return out


def run(in_maps, trace=False, **kwargs):
    from concourse.bass_utils import run_bass_kernel_spmd
    nc = get_nc()
    return run_bass_kernel_spmd(nc, in_maps, list(range(NCORES)),
                                trace=trace, **kwargs)


def kernel(q, k, v, Wq, bq, Wk, bk, Wv, bv, Wo, bo):
    in_maps = make_in_maps(q, k, v, Wq, bq, Wk, bk, Wv, bv, Wo, bo)
    res = run(in_maps)
    parts = [res.results[c]["out"] for c in range(NCORES)]
    return combine_outputs(parts, bo)


# revision 5
# speedup vs baseline: 1.5551x; 1.0822x over previous
"""Multi-head attention (Keras-style, relu-activated dense projections)
for Trainium2, SPMD across 8 NeuronCores.

Problem (full shapes):
    B, S, D, H = 4, 1024, 1024, 16 ; DH = 64
    qp = relu(q @ Wq + bq); kp = relu(k @ Wk + bk); vp = relu(v @ Wv + bv)
    per head h: scores = qh @ kh^T / 8 ; attn = softmax(scores)
    out = relu(concat_h(attn @ vh) @ Wo + bo)

Sharding: core c = (batch b = c//2, head-group g = c%2). Each core computes
the 8 heads of group g for batch b end-to-end and produces the partial
output projection  attn_out_g @ Wo[g*512:(g+1)*512, :]  (no bias / relu).
Host sums the two partials per batch, adds bo, applies relu.

v3 design notes:
  - x inputs and Wq/Wk/Wv in fp8 e4m3 (weights prescaled x16 on the host so
    they clear the e4m3 subnormal range; the 1/16 is folded into the ACT
    relu `scale` for Q/K and into the DVE (mult,max) tensor_scalar for V).
    Halves the input DMA, which bounds the kernel lead-in.
  - everything on-chip is bf16 (psum fp32).
  - unit U = (pc query-chunk, hp head-pair), pc-major order.  Scores of
    unit U are 8 ut-chunks (2 row-group-paired K=64 matmuls + exp each)
    with filler PE work interleaved so the PE never idles long enough for
    the HAM clock gate to re-throttle: remaining Q/K projections, the V
    projection, attn@V+Z of earlier units, the pc=0 output projection.
  - softmax denominator: exp ut-tiles are leaf-added pairwise on DVE as
    they appear (cascade), so only ~1.4us of summing remains after the
    last exp of a unit.  Z rows via two M=1 matmuls (partitions 0/32),
    masked K=33 broadcast matmul, reciprocal, one [128,512] multiply.
  - attn@V is gated per-ut on exp(U, ut) and column-pair packed (M=64).
"""

import numpy as np
from contextlib import ExitStack

import ml_dtypes

import concourse.bass as bass
import concourse.mybir as mybir
import concourse.tile as tile
from concourse import bacc

BF16NP = ml_dtypes.bfloat16
F8NP = ml_dtypes.float8_e4m3fn
WSCALE = 16.0

# ---- constants (hardcoded per the contract; kernel.py must be self-contained)
B, S, D, H = 4, 1024, 1024, 16
DG = 512          # feature slice per core (8 heads)
DH = 64
P = 128
NCORES = 8
NJT = DG // P     # 4 feature tiles == head pairs
NST = S // P      # 8 sequence tiles
NDT = D // P      # 8 contraction tiles for projections
NPC = S // 512    # 2 query chunks of 512

F32 = mybir.dt.float32
BF16 = mybir.dt.bfloat16
F8 = mybir.dt.float8e4
AF = mybir.ActivationFunctionType
ALU = mybir.AluOpType


def build_bass():
    nc = bacc.Bacc("TRN2", target_bir_lowering=False, debug=False,
                   num_devices=NCORES)

    xqT = nc.dram_tensor("xqT", [D, S], F8, kind="ExternalInput").ap()
    xkT = nc.dram_tensor("xkT", [D, S], F8, kind="ExternalInput").ap()
    xvT = nc.dram_tensor("xvT", [D, S], F8, kind="ExternalInput").ap()
    wq = nc.dram_tensor("wq", [D, DG], F8, kind="ExternalInput").ap()
    wk = nc.dram_tensor("wk", [D, DG], F8, kind="ExternalInput").ap()
    wv = nc.dram_tensor("wv", [D, DG], F8, kind="ExternalInput").ap()
    bq = nc.dram_tensor("bq", [1, DG], F32, kind="ExternalInput").ap()
    bk = nc.dram_tensor("bk", [1, DG], F32, kind="ExternalInput").ap()
    bv = nc.dram_tensor("bv", [1, DG], BF16, kind="ExternalInput").ap()
    wo = nc.dram_tensor("wo", [DG, D], BF16, kind="ExternalInput").ap()
    bcm_in = nc.dram_tensor("bcmask", [33, P], BF16, kind="ExternalInput").ap()
    out = nc.dram_tensor("out", [S, D], F32, kind="ExternalOutput").ap()

    # unit order: pc-major so the pc=0 output projection can start while
    # pc=1 attention is still draining.
    UNITS = [(pc, hp) for pc in range(NPC) for hp in range(NJT)]

    with tile.TileContext(nc) as tc, ExitStack() as ctx, \
            nc.allow_low_precision(reason="fp8/bf16 compute is intentional"):
        consts = ctx.enter_context(tc.tile_pool(name="consts", bufs=1))
        xpool = ctx.enter_context(tc.tile_pool(name="xpool", bufs=3))
        epool = ctx.enter_context(tc.tile_pool(name="epool", bufs=6))
        wpool = ctx.enter_context(tc.tile_pool(name="wpool", bufs=3))
        wopool = ctx.enter_context(tc.tile_pool(name="wopool", bufs=1))
        qkpool = ctx.enter_context(tc.tile_pool(name="qkpool", bufs=1))
        vpool = ctx.enter_context(tc.tile_pool(name="vpool", bufs=1))
        t1pool = ctx.enter_context(tc.tile_pool(name="t1pool", bufs=1))
        espool = ctx.enter_context(tc.tile_pool(name="espool", bufs=6))
        rpool = ctx.enter_context(tc.tile_pool(name="rpool", bufs=2))
        aopool = ctx.enter_context(tc.tile_pool(name="aopool", bufs=1))
        outpool = ctx.enter_context(tc.tile_pool(name="outpool", bufs=2))

        psA = ctx.enter_context(tc.tile_pool(name="psA", bufs=2, space="PSUM"))
        psP = ctx.enter_context(tc.tile_pool(name="psP", bufs=1, space="PSUM"))
        psB = ctx.enter_context(tc.tile_pool(name="psB", bufs=1, space="PSUM"))
        psZ = ctx.enter_context(tc.tile_pool(name="psZ", bufs=1, space="PSUM"))

        # --- constants (tiny DMAs / memsets first)
        bqT = consts.tile([P, NJT], F32, tag="bqT")
        nc.sync.dma_start(out=bqT, in_=bq[0, :].rearrange("(jt p) -> p jt", p=P))
        bkT = consts.tile([P, NJT], F32, tag="bkT")
        nc.sync.dma_start(out=bkT, in_=bk[0, :].rearrange("(jt p) -> p jt", p=P))
        bv_sb = consts.tile([1, DG], BF16, tag="bv")
        nc.sync.dma_start(out=bv_sb, in_=bv)
        bcmask = consts.tile([33, P], BF16, tag="bcmask")
        nc.sync.dma_start(out=bcmask, in_=bcm_in)
        onescol = consts.tile([P, 1], BF16, tag="onescol")
        nc.vector.memset(onescol, 1.0)
        onesrow = consts.tile([1, P], BF16, tag="onesrow")
        nc.vector.memset(onesrow, 1.0)
        # zsb: persistent Z staging rows (0 and 32); fill once with finite
        # values so the masked K=33 broadcast matmul never reads NaNs.
        zsb = consts.tile([33, 512], BF16, tag="zsb")
        nc.vector.memset(zsb, 1.0)

        # --- big input DMAs, in consumption order.  x tensors split in two
        # S-halves so the first projection matmuls can start earlier.
        def load_x(xT):
            xt = xpool.tile([P, NDT, S], F8, tag="x", name="x")
            for h in range(2):
                sl = slice(h * 512, (h + 1) * 512)
                nc.sync.dma_start(
                    out=xt[:, :, sl],
                    in_=xT[:, sl].rearrange("(dt p) s -> p dt s", p=P))
            return xt

        def load_w(w):
            wt = wpool.tile([P, NDT, DG], F8, tag="w", name="w")
            nc.sync.dma_start(out=wt, in_=w.rearrange("(dt p) n -> p dt n", p=P))
            return wt

        wqs = load_w(wq)
        xq = load_x(xqT)
        wks = load_w(wk)
        xk = load_x(xkT)
        wvs = load_w(wv)
        xv = load_x(xvT)
        wo3 = wopool.tile([P, NJT, D], BF16, tag="wo3")
        nc.sync.dma_start(out=wo3, in_=wo.rearrange("(hp p) n -> p hp n", p=P))

        qpT = qkpool.tile([P, NJT, S], BF16, tag="qpT")
        kpT = qkpool.tile([P, NJT, S], BF16, tag="kpT")
        vpa = vpool.tile([P, NST, DG], BF16, tag="vpa")
        aoT3 = aopool.tile([P, NJT, S], BF16, tag="aoT3")

        # ---- chunk emitters ------------------------------------------------
        def qk_half(which, hp, pc):
            """8 accumulating matmuls for one pc-half of qpT/kpT pair hp;
            emits the wide relu (with the 1/WSCALE fold) after pc 1."""
            xt, wt, bT, dst = ((xq, wqs, bqT, qpT) if which == "q"
                              else (xk, wks, bkT, kpT))
            if pc == 0:
                ps = qk_half.ps = psP.tile([P, 1024], F32, tag="ps", name="ps")
            else:
                ps = qk_half.ps
            half = ps[:, pc * 512:(pc + 1) * 512]
            for dt_ in range(NDT):
                nc.tensor.matmul(
                    half,
                    lhsT=wt[:, dt_, hp * P:(hp + 1) * P],
                    rhs=xt[:, dt_, pc * 512:(pc + 1) * 512],
                    start=(dt_ == 0), stop=(dt_ == NDT - 1))
            if pc == 1:
                nc.scalar.activation(dst[:, hp, :], ps, AF.Relu,
                                     bias=bT[:, hp:hp + 1], scale=1.0 / WSCALE)

        def v_chunk(st):
            """V projection for sequence tile st: 8 matmuls + bias matmul
            (bv prescaled x16 on host), relu+unscale on DVE."""
            ps = psP.tile([P, 1024], F32, tag="ps", name="ps")
            half = ps[:, 0:512]
            for dt_ in range(NDT):
                nc.tensor.matmul(
                    half,
                    lhsT=xv[:, dt_, st * P:(st + 1) * P],
                    rhs=wvs[:, dt_, :],
                    start=(dt_ == 0), stop=False)
            nc.tensor.matmul(half, lhsT=onesrow, rhs=bv_sb,
                             start=False, stop=True)
            nc.vector.tensor_scalar(vpa[:, st, :], half, 1.0 / WSCALE, 0.0,
                                    ALU.mult, ALU.max)

        ex_tiles = {}
        lf_tiles = {}
        exsums = {}

        def sc_chunk(u, ut):
            """Scores ut-tile of unit u: 2 concurrent K=64 matmuls (head
            pair in row groups 0:64 / 64:128), exp, and the cascaded DVE
            leaf-sum toward the softmax denominator."""
            pc, hp = UNITS[u]
            if ut == 0:
                ex_tiles[u] = epool.tile([P, NST, 1024], BF16, tag="ex",
                                         name="ex")
            ex = ex_tiles[u]
            pslice = slice(pc * 512, (pc + 1) * 512)
            uslice = slice(ut * P, (ut + 1) * P)
            pw = psA.tile([P, 1024], F32, tag="ps", name="pw")
            nc.tensor.matmul(
                pw[:, 0:512],
                lhsT=kpT[0:DH, hp, uslice],
                rhs=qpT[0:DH, hp, pslice],
                start=True, stop=True)
            nc.tensor.matmul(
                pw[:, 512:1024],
                lhsT=kpT[DH:P, hp, uslice],
                rhs=qpT[DH:P, hp, pslice],
                start=True, stop=True)
            nc.scalar.activation(ex[:, ut, :], pw, AF.Exp, scale=0.125)
            if ut % 2 == 1:
                j = ut // 2
                if j == 0:
                    lf_tiles[u] = t1pool.tile([P, 2, 1024], BF16, tag="lf",
                                              name="lf")
                lf = lf_tiles[u]
                dst = lf[:, 0, :] if j == 0 else lf[:, 1, :]
                nc.vector.tensor_add(dst, ex[:, ut - 1, :], ex[:, ut, :])
                if j in (1, 2):
                    nc.vector.tensor_add(lf[:, 0, :], lf[:, 0, :], lf[:, 1, :])
                elif j == 3:
                    exsum = espool.tile([P, 1024], BF16, tag="exsum",
                                        name="exsum")
                    nc.vector.tensor_add(exsum, lf[:, 0, :], lf[:, 1, :])
                    exsums[u] = exsum

        nt_tiles = {}

        def av_chunk(u, uts):
            """attn@V for unit u over ut in uts: column-pair-packed bf16
            matmuls accumulating into nt (head A rows 0:64, head B 64:128)."""
            pc, hp = UNITS[u]
            hA, hB = 2 * hp, 2 * hp + 1
            ex = ex_tiles[u]
            if uts[0] == 0:
                nt_tiles[u] = psB.tile([P, 512], F32, tag="nt", name="nt")
            nt = nt_tiles[u]
            for ut in uts:
                nc.tensor.matmul(
                    nt[0:DH, :],
                    lhsT=vpa[:, ut, hA * DH:(hA + 1) * DH],
                    rhs=ex[:, ut, 0:512],
                    start=(ut == 0), stop=(ut == NST - 1),
                    skip_group_check=True)
                nc.tensor.matmul(
                    nt[DH:P, :],
                    lhsT=vpa[:, ut, hB * DH:(hB + 1) * DH],
                    rhs=ex[:, ut, 512:1024],
                    start=(ut == 0), stop=(ut == NST - 1),
                    skip_group_check=True)

        def z_chunk(u):
            """Z reduction + broadcast + reciprocal + normalize for unit u:
            writes aoT3[:, hp, pslice]."""
            pc, hp = UNITS[u]
            pslice = slice(pc * 512, (pc + 1) * 512)
            nt = nt_tiles.pop(u)
            exsum = exsums.pop(u)
            zps = psZ.tile([P, 512], F32, tag="z", name="zps")
            nc.tensor.matmul(zps[0:1, :], lhsT=onescol,
                             rhs=exsum[:, 0:512], start=True, stop=True)
            nc.tensor.matmul(zps[32:33, :], lhsT=onescol,
                             rhs=exsum[:, 512:1024], start=True, stop=True)
            nc.vector.tensor_copy(zsb[0:1, :], zps[0:1, :])
            nc.vector.tensor_copy(zsb[32:33, :], zps[32:33, :])
            zbc = psZ.tile([P, 512], F32, tag="z", name="zbc")
            nc.tensor.matmul(zbc, lhsT=bcmask, rhs=zsb, start=True, stop=True)
            rcp = rpool.tile([P, 512], F32, tag="rcp", name="rcp")
            nc.vector.reciprocal_approx_fast(rcp, zbc)
            nc.vector.tensor_mul(aoT3[:, hp, pslice], nt, rcp)
            del ex_tiles[u]

        def outp_chunk(pt, drain_on_act=False):
            """Output projection for query tile pt: 2x4 accumulating
            matmuls, one wide copy, one DMA."""
            po = psP.tile([P, 1024], F32, tag="ps", name="po")
            for jj in range(2):
                for hp in range(NJT):
                    nc.tensor.matmul(
                        po[:, jj * 512:(jj + 1) * 512],
                        lhsT=aoT3[:, hp, pt * P:(pt + 1) * P],
                        rhs=wo3[:, hp, jj * 512:(jj + 1) * 512],
                        start=(hp == 0), stop=(hp == NJT - 1))
            os_ = outpool.tile([P, 1024], F32, tag="os", name="os")
            if drain_on_act:
                nc.scalar.copy(os_, po)
            else:
                nc.vector.tensor_copy(os_, po)
            nc.sync.dma_start(out=out[pt * P:(pt + 1) * P, :], in_=os_)

        # ---- emission schedule --------------------------------------------
        def stretch(u, fillers):
            """Scores unit u (8 ut chunks) with filler chunks interleaved
            2 score-chunks : 1 filler."""
            fi = iter(fillers)
            for ut in range(NST):
                sc_chunk(u, ut)
                if ut % 2 == 1:
                    f = next(fi, None)
                    if f is not None:
                        f()
            for f in fi:
                f()

        # head-pair 0 projections first (nothing to overlap them with)
        for pc in range(NPC):
            qk_half("q", 0, pc)
        for pc in range(NPC):
            qk_half("k", 0, pc)

        # U0..U3 = pc0 units; U4..U7 = pc1 units
        stretch(0, [lambda: qk_half("q", 1, 0), lambda: qk_half("q", 1, 1),
                    lambda: qk_half("k", 1, 0), lambda: qk_half("k", 1, 1)])
        stretch(1, [lambda: qk_half("q", 2, 0), lambda: qk_half("q", 2, 1),
                    lambda: qk_half("k", 2, 0), lambda: qk_half("k", 2, 1)])
        stretch(2, [lambda: qk_half("q", 3, 0), lambda: qk_half("q", 3, 1),
                    lambda: qk_half("k", 3, 0), lambda: qk_half("k", 3, 1)])
        stretch(3, [lambda: v_chunk(0), lambda: v_chunk(1),
                    lambda: v_chunk(2), lambda: v_chunk(3)])
        stretch(4, [lambda: v_chunk(4), lambda: v_chunk(5),
                    lambda: v_chunk(6), lambda: v_chunk(7)])
        stretch(5, [lambda: av_chunk(0, (0, 1)), lambda: av_chunk(0, (2, 3)),
                    lambda: av_chunk(0, (4, 5)), lambda: av_chunk(0, (6, 7)),
                    lambda: z_chunk(0),
                    lambda: av_chunk(1, (0, 1)), lambda: av_chunk(1, (2, 3))])
        stretch(6, [lambda: av_chunk(1, (4, 5)), lambda: av_chunk(1, (6, 7)),
                    lambda: z_chunk(1),
                    lambda: av_chunk(2, (0, 1)), lambda: av_chunk(2, (2, 3)),
                    lambda: av_chunk(2, (4, 5)), lambda: av_chunk(2, (6, 7)),
                    lambda: z_chunk(2)])
        stretch(7, [lambda: av_chunk(3, (0, 1)), lambda: av_chunk(3, (2, 3)),
                    lambda: av_chunk(3, (4, 5)), lambda: av_chunk(3, (6, 7)),
                    lambda: z_chunk(3),
                    lambda: av_chunk(4, (0, 1)), lambda: av_chunk(4, (2, 3))])

        # tail: remaining attn@V units zipped with the output projections.
        av_chunk(4, (4, 5))
        av_chunk(4, (6, 7))
        z_chunk(4)
        outp_chunk(0)
        av_chunk(5, (0, 1, 2, 3))
        av_chunk(5, (4, 5, 6, 7))
        z_chunk(5)
        outp_chunk(1)
        av_chunk(6, (0, 1, 2, 3))
        av_chunk(6, (4, 5, 6, 7))
        z_chunk(6)
        outp_chunk(2)
        av_chunk(7, (0, 1, 2, 3))
        av_chunk(7, (4, 5, 6, 7))
        z_chunk(7)
        outp_chunk(3)
        for pt in range(4):
            outp_chunk(4 + pt, drain_on_act=True)

    nc.compile()
    return nc


_CACHE = {}


def get_nc():
    if "nc" not in _CACHE:
        _CACHE["nc"] = build_bass()
    return _CACHE["nc"]


def make_bcmask():
    m = np.zeros((33, P), np.float32)
    m[0, 0:DH] = 1.0
    m[32, DH:P] = 1.0
    return m.astype(BF16NP)


def make_in_maps(q, k, v, Wq, bq, Wk, bk, Wv, bv, Wo, bo):
    q = np.asarray(q, np.float32)
    k = np.asarray(k, np.float32)
    v = np.asarray(v, np.float32)
    Wq = np.asarray(Wq, np.float32) * WSCALE
    Wk = np.asarray(Wk, np.float32) * WSCALE
    Wv = np.asarray(Wv, np.float32) * WSCALE
    Wo = np.asarray(Wo, np.float32)
    bq = np.asarray(bq, np.float32)
    bk = np.asarray(bk, np.float32)
    bv = np.asarray(bv, np.float32) * WSCALE

    qT = [np.ascontiguousarray(q[b].T).astype(F8NP) for b in range(B)]
    kT = [np.ascontiguousarray(k[b].T).astype(F8NP) for b in range(B)]
    vT = [np.ascontiguousarray(v[b].T).astype(F8NP) for b in range(B)]
    bcm = make_bcmask()

    in_maps = []
    for c in range(NCORES):
        b, g = divmod(c, 2)
        sl = slice(g * DG, (g + 1) * DG)
        in_maps.append({
            "xqT": qT[b],
            "xkT": kT[b],
            "xvT": vT[b],
            "wq": np.ascontiguousarray(Wq[:, sl]).astype(F8NP),
            "wk": np.ascontiguousarray(Wk[:, sl]).astype(F8NP),
            "wv": np.ascontiguousarray(Wv[:, sl]).astype(F8NP),
            "bq": np.ascontiguousarray(bq[sl]).reshape(1, DG),
            "bk": np.ascontiguousarray(bk[sl]).reshape(1, DG),
            "bv": np.ascontiguousarray(bv[sl]).reshape(1, DG).astype(BF16NP),
            "wo": np.ascontiguousarray(Wo[sl, :]).astype(BF16NP),
            "bcmask": bcm,
        })
    return in_maps


def combine_outputs(parts, bo):
    bo = np.asarray(bo, np.float32)
    out = np.empty((B, S, D), np.float32)
    for b in range(B):
        out[b] = np.maximum(
            np.asarray(parts[2 * b], np.float32)
            + np.asarray(parts[2 * b + 1], np.float32) + bo[None, :], 0.0)
    return out


def run(in_maps, trace=False, **kwargs):
    from concourse.bass_utils import run_bass_kernel_spmd
    nc = get_nc()
    return run_bass_kernel_spmd(nc, in_maps, list(range(NCORES)),
                                trace=trace, **kwargs)


def kernel(q, k, v, Wq, bq, Wk, bk, Wv, bv, Wo, bo):
    in_maps = make_in_maps(q, k, v, Wq, bq, Wk, bk, Wv, bv, Wo, bo)
    res = run(in_maps)
    parts = [res.results[c]["out"] for c in range(NCORES)]
    return combine_outputs(parts, bo)


# revision 7
# speedup vs baseline: 1.5663x; 1.0072x over previous
"""Multi-head attention (Keras-style, relu-activated dense projections)
for Trainium2, SPMD across 8 NeuronCores.

Problem (full shapes):
    B, S, D, H = 4, 1024, 1024, 16 ; DH = 64
    qp = relu(q @ Wq + bq); kp = relu(k @ Wk + bk); vp = relu(v @ Wv + bv)
    per head h: scores = qh @ kh^T / 8 ; attn = softmax(scores)
    out = relu(concat_h(attn @ vh) @ Wo + bo)

Sharding: core c = (batch b = c//2, head-group g = c%2). Each core computes
the 8 heads of group g for batch b end-to-end and produces the partial
output projection  attn_out_g @ Wo[g*512:(g+1)*512, :]  (no bias / relu).
Host sums the two partials per batch, adds bo, applies relu.

v4 design notes:
  - x inputs and Wq/Wk/Wv in fp8 e4m3 (weights prescaled x16 on the host
    so they clear the e4m3 subnormal range; the 1/16 is folded into the
    ACT relu `scale` for Q/K and the DVE (mult,max) tensor_scalar for V).
  - all DMA'd tensors are host-packed so SBUF partition p reads one
    contiguous DRAM row (128 descriptors/transfer, multi-KB inner) - the
    strided-rearrange version cost 2-3us of descriptor generation per
    transfer and ran below line rate.
  - unit U = (pc query-chunk, hp head-pair), pc-major.  Scores of unit U
    are 8 ut-chunks (2 row-group-paired K=64 matmuls + exp each) with one
    ~4-matmul filler chunk interleaved after every ut: remaining Q/K
    projection halves, V projection halves, attn@V of earlier units, Z
    chunks, the output projection.  This keeps the PE busy through the
    ACT-paced stretches so the HAM clock gate never re-throttles.
  - softmax denominator: exp ut-tiles leaf-added pairwise on DVE as they
    appear (cascade) so ~1.4us of summing remains after a unit's last
    exp.  z_pre (Z matmuls M=1 at partitions 0/32 -> strided copy ->
    masked K=33 broadcast matmul -> reciprocal) runs as early filler;
    z_fin is only the final [128,512] multiply.
  - attn@V gated per-ut on exp(U, ut), column-pair packed (M=64).
"""

import numpy as np
from contextlib import ExitStack

import ml_dtypes

import concourse.bass as bass
import concourse.mybir as mybir
import concourse.tile as tile
from concourse import bacc

BF16NP = ml_dtypes.bfloat16
F8NP = ml_dtypes.float8_e4m3fn
WSCALE = 16.0

# ---- constants (hardcoded per the contract; kernel.py must be self-contained)
B, S, D, H = 4, 1024, 1024, 16
DG = 512          # feature slice per core (8 heads)
DH = 64
P = 128
NCORES = 8
NJT = DG // P     # 4 feature tiles == head pairs
NST = S // P      # 8 sequence tiles
NDT = D // P      # 8 contraction tiles for projections
NPC = S // 512    # 2 query chunks of 512

F32 = mybir.dt.float32
BF16 = mybir.dt.bfloat16
F8 = mybir.dt.float8e4
AF = mybir.ActivationFunctionType
ALU = mybir.AluOpType


def build_bass():
    nc = bacc.Bacc("TRN2", target_bir_lowering=False, debug=False,
                   num_devices=NCORES)

    # host-packed layouts: partition-major, contiguous free dim
    xqT = nc.dram_tensor("xqT", [P, NDT, S], F8, kind="ExternalInput").ap()
    xkT = nc.dram_tensor("xkT", [P, NDT, S], F8, kind="ExternalInput").ap()
    xvT = nc.dram_tensor("xvT", [P, NDT, S], F8, kind="ExternalInput").ap()
    wq = nc.dram_tensor("wq", [P, NDT, DG], F8, kind="ExternalInput").ap()
    wk = nc.dram_tensor("wk", [P, NDT, DG], F8, kind="ExternalInput").ap()
    wv = nc.dram_tensor("wv", [P, NDT, DG], F8, kind="ExternalInput").ap()
    wo = nc.dram_tensor("wo", [P, NJT, D], BF16, kind="ExternalInput").ap()
    bq = nc.dram_tensor("bq", [P, NJT], F32, kind="ExternalInput").ap()
    bk = nc.dram_tensor("bk", [P, NJT], F32, kind="ExternalInput").ap()
    bv = nc.dram_tensor("bv", [1, DG], BF16, kind="ExternalInput").ap()
    bcm_in = nc.dram_tensor("bcmask", [33, P], BF16, kind="ExternalInput").ap()
    out = nc.dram_tensor("out", [S, D], F32, kind="ExternalOutput").ap()

    # unit order: pc-major so the pc=0 output projection can start while
    # pc=1 attention is still draining.
    UNITS = [(pc, hp) for pc in range(NPC) for hp in range(NJT)]

    with tile.TileContext(nc) as tc, ExitStack() as ctx, \
            nc.allow_low_precision(reason="fp8/bf16 compute is intentional"):
        consts = ctx.enter_context(tc.tile_pool(name="consts", bufs=1))
        xpool = ctx.enter_context(tc.tile_pool(name="xpool", bufs=3))
        epool = ctx.enter_context(tc.tile_pool(name="epool", bufs=6))
        wpool = ctx.enter_context(tc.tile_pool(name="wpool", bufs=3))
        wopool = ctx.enter_context(tc.tile_pool(name="wopool", bufs=1))
        qkpool = ctx.enter_context(tc.tile_pool(name="qkpool", bufs=1))
        vpool = ctx.enter_context(tc.tile_pool(name="vpool", bufs=1))
        t1pool = ctx.enter_context(tc.tile_pool(name="t1pool", bufs=1))
        espool = ctx.enter_context(tc.tile_pool(name="espool", bufs=6))
        rpool = ctx.enter_context(tc.tile_pool(name="rpool", bufs=2))
        aopool = ctx.enter_context(tc.tile_pool(name="aopool", bufs=1))
        outpool = ctx.enter_context(tc.tile_pool(name="outpool", bufs=2))

        psA = ctx.enter_context(tc.tile_pool(name="psA", bufs=2, space="PSUM"))
        psP = ctx.enter_context(tc.tile_pool(name="psP", bufs=1, space="PSUM"))
        psB = ctx.enter_context(tc.tile_pool(name="psB", bufs=1, space="PSUM"))
        psZ = ctx.enter_context(tc.tile_pool(name="psZ", bufs=1, space="PSUM"))

        # --- big input DMAs first (the first matmuls gate on them); small
        # const DMAs go after wq..xk so they don't head-block the queue.
        def load3(pool, dram, shape, dt, tag):
            t = pool.tile(shape, dt, tag=tag, name=tag)
            nc.sync.dma_start(out=t, in_=dram)
            return t

        wqs = load3(wpool, wq, [P, NDT, DG], F8, "w")
        xq = load3(xpool, xqT, [P, NDT, S], F8, "x")
        wks = load3(wpool, wk, [P, NDT, DG], F8, "w")
        xk = load3(xpool, xkT, [P, NDT, S], F8, "x")

        bqT = consts.tile([P, NJT], F32, tag="bqT")
        nc.sync.dma_start(out=bqT, in_=bq)
        bkT = consts.tile([P, NJT], F32, tag="bkT")
        nc.sync.dma_start(out=bkT, in_=bk)
        bv_sb = consts.tile([1, DG], BF16, tag="bv")
        nc.sync.dma_start(out=bv_sb, in_=bv)
        bcmask = consts.tile([33, P], BF16, tag="bcmask")
        nc.sync.dma_start(out=bcmask, in_=bcm_in)
        onescol = consts.tile([P, 1], BF16, tag="onescol")
        nc.vector.memset(onescol, 1.0)
        onesrow = consts.tile([1, P], BF16, tag="onesrow")
        nc.vector.memset(onesrow, 1.0)
        # zsb: persistent Z staging rows (0 and 32); fill once with finite
        # values so the masked K=33 broadcast matmul never reads NaNs.
        zsb = consts.tile([33, 512], BF16, tag="zsb")
        nc.vector.memset(zsb, 1.0)

        wvs = load3(wpool, wv, [P, NDT, DG], F8, "w")
        xv = load3(xpool, xvT, [P, NDT, S], F8, "x")
        wo3 = load3(wopool, wo, [P, NJT, D], BF16, "wo3")

        qpT = qkpool.tile([P, NJT, S], BF16, tag="qpT")
        kpT = qkpool.tile([P, NJT, S], BF16, tag="kpT")
        vpa = vpool.tile([P, NST, DG], BF16, tag="vpa")
        aoT3 = aopool.tile([P, NJT, S], BF16, tag="aoT3")

        # ---- chunk emitters ------------------------------------------------
        qk_ps = {}

        def qk_sub(which, hp, pc, h2):
            """4 accumulating matmuls (dt h2*4..h2*4+3) for one pc-half of
            qpT/kpT pair hp; relu+bias+unscale after the last quarter."""
            xt, wt, bT, dst = ((xq, wqs, bqT, qpT) if which == "q"
                              else (xk, wks, bkT, kpT))
            if pc == 0 and h2 == 0:
                qk_ps[which] = psP.tile([P, 1024], F32, tag="ps", name="ps")
            ps = qk_ps[which]
            half = ps[:, pc * 512:(pc + 1) * 512]
            for dt_ in range(h2 * 4, h2 * 4 + 4):
                nc.tensor.matmul(
                    half,
                    lhsT=wt[:, dt_, hp * P:(hp + 1) * P],
                    rhs=xt[:, dt_, pc * 512:(pc + 1) * 512],
                    start=(dt_ == 0), stop=(dt_ == NDT - 1))
            if pc == 1 and h2 == 1:
                nc.scalar.activation(dst[:, hp, :], ps, AF.Relu,
                                     bias=bT[:, hp:hp + 1], scale=1.0 / WSCALE)

        v_ps = {}

        def v_sub(st, h2):
            """V projection half for sequence tile st; bias matmul (bv
            prescaled x16 on host) + DVE relu/unscale after the last half."""
            if h2 == 0:
                v_ps[st] = psP.tile([P, 1024], F32, tag="ps", name="ps")
            half = v_ps[st][:, 0:512]
            for dt_ in range(h2 * 4, h2 * 4 + 4):
                nc.tensor.matmul(
                    half,
                    lhsT=xv[:, dt_, st * P:(st + 1) * P],
                    rhs=wvs[:, dt_, :],
                    start=(dt_ == 0), stop=False)
            if h2 == 1:
                nc.tensor.matmul(half, lhsT=onesrow, rhs=bv_sb,
                                 start=False, stop=True)
                nc.vector.tensor_scalar(vpa[:, st, :], half, 1.0 / WSCALE,
                                        0.0, ALU.mult, ALU.max)
                del v_ps[st]

        ex_tiles = {}
        lf_tiles = {}
        exsums = {}

        def sc_chunk(u, ut):
            """Scores ut-tile of unit u: 2 concurrent K=64 matmuls (head
            pair in row groups 0:64 / 64:128), exp, and the cascaded DVE
            leaf-sum toward the softmax denominator."""
            pc, hp = UNITS[u]
            if ut == 0:
                ex_tiles[u] = epool.tile([P, NST, 1024], BF16, tag="ex",
                                         name="ex")
            ex = ex_tiles[u]
            pslice = slice(pc * 512, (pc + 1) * 512)
            uslice = slice(ut * P, (ut + 1) * P)
            pw = psA.tile([P, 1024], F32, tag="ps", name="pw")
            nc.tensor.matmul(
                pw[:, 0:512],
                lhsT=kpT[0:DH, hp, uslice],
                rhs=qpT[0:DH, hp, pslice],
                start=True, stop=True)
            nc.tensor.matmul(
                pw[:, 512:1024],
                lhsT=kpT[DH:P, hp, uslice],
                rhs=qpT[DH:P, hp, pslice],
                start=True, stop=True)
            nc.scalar.activation(ex[:, ut, :], pw, AF.Exp, scale=0.125)
            if ut % 2 == 1:
                j = ut // 2
                if j == 0:
                    lf_tiles[u] = t1pool.tile([P, 2, 1024], BF16, tag="lf",
                                              name="lf")
                lf = lf_tiles[u]
                dst = lf[:, 0, :] if j == 0 else lf[:, 1, :]
                nc.vector.tensor_add(dst, ex[:, ut - 1, :], ex[:, ut, :])
                if j in (1, 2):
                    nc.vector.tensor_add(lf[:, 0, :], lf[:, 0, :], lf[:, 1, :])
                elif j == 3:
                    exsum = espool.tile([P, 1024], BF16, tag="exsum",
                                        name="exsum")
                    nc.vector.tensor_add(exsum, lf[:, 0, :], lf[:, 1, :])
                    exsums[u] = exsum

        nt_tiles = {}

        def av_chunk(u, uts):
            """attn@V for unit u over ut in uts: column-pair-packed bf16
            matmuls accumulating into nt (head A rows 0:64, head B 64:128)."""
            pc, hp = UNITS[u]
            hA, hB = 2 * hp, 2 * hp + 1
            ex = ex_tiles[u]
            if uts[0] == 0:
                nt_tiles[u] = psB.tile([P, 512], F32, tag="nt", name="nt")
            nt = nt_tiles[u]
            for ut in uts:
                nc.tensor.matmul(
                    nt[0:DH, :],
                    lhsT=vpa[:, ut, hA * DH:(hA + 1) * DH],
                    rhs=ex[:, ut, 0:512],
                    start=(ut == 0), stop=(ut == NST - 1),
                    skip_group_check=True)
                nc.tensor.matmul(
                    nt[DH:P, :],
                    lhsT=vpa[:, ut, hB * DH:(hB + 1) * DH],
                    rhs=ex[:, ut, 512:1024],
                    start=(ut == 0), stop=(ut == NST - 1),
                    skip_group_check=True)

        rcps = {}

        def z_pre(u):
            """Z reduction + broadcast + reciprocal for unit u (no nt dep,
            so it can run as early filler)."""
            exsum = exsums.pop(u)
            zps = psZ.tile([P, 512], F32, tag="z", name="zps")
            nc.tensor.matmul(zps[0:1, :], lhsT=onescol,
                             rhs=exsum[:, 0:512], start=True, stop=True)
            nc.tensor.matmul(zps[32:33, :], lhsT=onescol,
                             rhs=exsum[:, 512:1024], start=True, stop=True)
            nc.vector.tensor_copy(zsb[0:1, :], zps[0:1, :])
            nc.vector.tensor_copy(zsb[32:33, :], zps[32:33, :])
            zbc = psZ.tile([P, 512], F32, tag="z", name="zbc")
            nc.tensor.matmul(zbc, lhsT=bcmask, rhs=zsb, start=True, stop=True)
            rcp = rpool.tile([P, 512], F32, tag="rcp", name="rcp")
            nc.vector.reciprocal_approx_fast(rcp, zbc)
            rcps[u] = rcp

        def z_fin(u):
            """Normalize attn@V of unit u into aoT3 (one DVE multiply)."""
            pc, hp = UNITS[u]
            pslice = slice(pc * 512, (pc + 1) * 512)
            nc.vector.tensor_mul(aoT3[:, hp, pslice], nt_tiles.pop(u),
                                 rcps.pop(u))
            del ex_tiles[u]

        def outp_chunk(pt, pool=None, drain_on_act=False):
            """Output projection for query tile pt: 2x4 accumulating
            matmuls, one wide copy, one DMA."""
            po = (pool or psP).tile([P, 1024], F32, tag="ps", name="po")
            for jj in range(2):
                for hp in range(NJT):
                    nc.tensor.matmul(
                        po[:, jj * 512:(jj + 1) * 512],
                        lhsT=aoT3[:, hp, pt * P:(pt + 1) * P],
                        rhs=wo3[:, hp, jj * 512:(jj + 1) * 512],
                        start=(hp == 0), stop=(hp == NJT - 1))
            os_ = outpool.tile([P, 1024], F32, tag="os", name="os")
            if drain_on_act:
                nc.scalar.copy(os_, po)
            else:
                nc.vector.tensor_copy(os_, po)
            nc.sync.dma_start(out=out[pt * P:(pt + 1) * P, :], in_=os_)

        # ---- emission schedule --------------------------------------------
        def stretch(u, fillers):
            """Scores unit u (8 ut chunks) with one filler chunk after
            every ut."""
            fi = iter(fillers)
            for ut in range(NST):
                sc_chunk(u, ut)
                f = next(fi, None)
                if f is not None:
                    f()
            for f in fi:
                f()

        def qk8(hp):
            return [lambda w=w, pc=pc, h2=h2: qk_sub(w, hp, pc, h2)
                    for w in ("q", "k") for pc in range(2) for h2 in range(2)]

        def v8(st0):
            return [lambda st=st, h2=h2: v_sub(st, h2)
                    for st in range(st0, st0 + 4) for h2 in range(2)]

        def drain6(u):
            return [lambda: z_pre(u),
                    lambda: av_chunk(u, (0, 1)), lambda: av_chunk(u, (2, 3)),
                    lambda: av_chunk(u, (4, 5)), lambda: av_chunk(u, (6, 7)),
                    lambda: z_fin(u)]

        # head-pair 0 projections first (nothing to overlap them with)
        for f in qk8(0):
            f()

        # U0..U3 = pc0 units; U4..U7 = pc1 units
        stretch(0, qk8(1))
        stretch(1, qk8(2))
        stretch(2, qk8(3))
        stretch(3, v8(0))
        stretch(4, v8(4))
        d0, d1, d2, d3 = drain6(0), drain6(1), drain6(2), drain6(3)
        stretch(5, d0 + d1[:2])
        stretch(6, d1[2:] + d2[:4])
        stretch(7, d2[4:] + d3)

        # tail: pc1 unit drains zipped with the output projections.
        for f in drain6(4):
            f()
        outp_chunk(0)
        for f in drain6(5):
            f()
        outp_chunk(1)
        for f in drain6(6):
            f()
        outp_chunk(2)
        for f in drain6(7):
            f()
        outp_chunk(3)
        outp_chunk(4, drain_on_act=True)
        outp_chunk(5, pool=psA)
        outp_chunk(6, drain_on_act=True)
        outp_chunk(7, pool=psA)

    nc.compile()
    return nc


_CACHE = {}


def get_nc():
    if "nc" not in _CACHE:
        _CACHE["nc"] = build_bass()
    return _CACHE["nc"]


def make_bcmask():
    m = np.zeros((33, P), np.float32)
    m[0, 0:DH] = 1.0
    m[32, DH:P] = 1.0
    return m.astype(BF16NP)


def pack_rows(a, nd):
    """[nd*128, N] -> [128, nd, N] partition-major contiguous."""
    n = a.shape[1]
    return np.ascontiguousarray(
        a.reshape(nd, P, n).transpose(1, 0, 2))


def make_in_maps(q, k, v, Wq, bq, Wk, bk, Wv, bv, Wo, bo):
    q = np.asarray(q, np.float32)
    k = np.asarray(k, np.float32)
    v = np.asarray(v, np.float32)
    Wq = np.asarray(Wq, np.float32) * WSCALE
    Wk = np.asarray(Wk, np.float32) * WSCALE
    Wv = np.asarray(Wv, np.float32) * WSCALE
    Wo = np.asarray(Wo, np.float32)
    bq = np.asarray(bq, np.float32)
    bk = np.asarray(bk, np.float32)
    bv = np.asarray(bv, np.float32) * WSCALE

    qT = [pack_rows(q[b].T.astype(F8NP), NDT) for b in range(B)]
    kT = [pack_rows(k[b].T.astype(F8NP), NDT) for b in range(B)]
    vT = [pack_rows(v[b].T.astype(F8NP), NDT) for b in range(B)]
    bcm = make_bcmask()

    in_maps = []
    for c in range(NCORES):
        b, g = divmod(c, 2)
        sl = slice(g * DG, (g + 1) * DG)
        in_maps.append({
            "xqT": qT[b],
            "xkT": kT[b],
            "xvT": vT[b],
            "wq": pack_rows(Wq[:, sl].astype(F8NP), NDT),
            "wk": pack_rows(Wk[:, sl].astype(F8NP), NDT),
            "wv": pack_rows(Wv[:, sl].astype(F8NP), NDT),
            "bq": np.ascontiguousarray(bq[sl]).reshape(NJT, P).T.copy(),
            "bk": np.ascontiguousarray(bk[sl]).reshape(NJT, P).T.copy(),
            "bv": np.ascontiguousarray(bv[sl]).reshape(1, DG).astype(BF16NP),
            "wo": pack_rows(Wo[sl, :].astype(BF16NP), NJT),
            "bcmask": bcm,
        })
    return in_maps


def combine_outputs(parts, bo):
    bo = np.asarray(bo, np.float32)
    out = np.empty((B, S, D), np.float32)
    for b in range(B):
        out[b] = np.maximum(
            np.asarray(parts[2 * b], np.float32)
            + np.asarray(parts[2 * b + 1], np.float32) + bo[None, :], 0.0)
    return out


def run(in_maps, trace=False, **kwargs):
    from concourse.bass_utils import run_bass_kernel_spmd
    nc = get_nc()
    return run_bass_kernel_spmd(nc, in_maps, list(range(NCORES)),
                                trace=trace, **kwargs)


def kernel(q, k, v, Wq, bq, Wk, bk, Wv, bv, Wo, bo):
    in_maps = make_in_maps(q, k, v, Wq, bq, Wk, bk, Wv, bv, Wo, bo)
    res = run(in_maps)
    parts = [res.results[c]["out"] for c in range(NCORES)]
    return combine_outputs(parts, bo)
